# revision 15
# baseline (speedup 1.0000x reference)
"""Trainium2 Bass kernel for nn_MultiHeadAttention_59992103190912.

Strategy (8 cores): data-parallel over batch (4) x tensor-parallel over heads
(2-way, 8 heads/core).  Each core computes, for its (batch b, head-half hh):

    xs   = x^T * rstd(t)                      (LayerNorm folded into weights)
    Q^T  = Wq_aug^T @ [xs; c; 1]              [512 feat, T]   (c = -mu*rstd)
    K^T  = Wk_aug^T @ [xs; c; 1]              [512 feat, T]
    V    = [xs; c; 1]^T @ Wv_aug              [T, 512 feat]   (+ ones column/head)
    S^T  = K^T_h K-tiles  x  Q^T_h            [k, q] layout per head
    Pu   = exp(SCALE * (S^T .* mask^T))       multiplicative mask, DVE mul + ACT exp
    OT_u = V_ext^T-chunks @ Pu                [65, q]; row 64 = softmax denom
    avT  = OT_u[0:64] * exp(-ln(den))         division-free normalize
    out_p= avT^T @ Wo_half                    [T, D] partial (host sums pair + bo)

All activation functions used (exp, ln, square) live in one ACT table set
(natural_log_exp_and_others) so there are no table reloads in the hot loop.
"""

import sys

sys.path.insert(0, "/opt/trn_rl_repo")

from contextlib import ExitStack
from dataclasses import dataclass

import numpy as np

import concourse.bass as bass
import concourse.tile as tile
from concourse import mybir

F32 = mybir.dt.float32
AF = mybir.ActivationFunctionType
ALU = mybir.AluOpType


@dataclass(frozen=True)
class Dims:
    T: int = 2048      # sequence length
    D: int = 1024      # model dim
    HL: int = 8        # heads per core
    DH: int = 64       # head dim
    QC: int = 512      # q-chunk width for attention phase
    LN_EPS: float = 1e-5

    @property
    def F(self):       # features per core (= HL * DH)
        return self.HL * self.DH

    @property
    def ND(self):      # d-chunks of 128
        return self.D // 128

    @property
    def NF(self):      # feature tiles of 128
        return self.F // 128

    @property
    def NKT(self):     # k tiles of 128
        return self.T // 128

    @property
    def NQC(self):     # q chunks
        return self.T // self.QC

    @property
    def NTC(self):     # 512-wide t-chunks (QKV phase)
        return self.T // 512

    @property
    def SCALE(self):
        return self.DH ** -0.5


def build_bass(d: Dims = Dims(), dbg: str | None = None) -> bass.Bass:
    assert d.T % 512 == 0 and d.D % 128 == 0 and d.F % 128 == 0
    assert d.QC % 512 == 0 and d.T % d.QC == 0
    nc = bass.Bass()

    xT_d = nc.declare_dram_parameter("xT", [d.D, d.T], F32, isOutput=False)
    mk_d = nc.declare_dram_parameter("maskT", [d.T, d.T], F32, isOutput=False)
    wq_d = nc.declare_dram_parameter("Wq", [d.D + 2, d.F], F32, isOutput=False)
    wk_d = nc.declare_dram_parameter("Wk", [d.D + 2, d.F], F32, isOutput=False)
    wv_d = nc.declare_dram_parameter("Wv", [d.D + 2, d.F], F32, isOutput=False)
    wo_d = nc.declare_dram_parameter("Wo", [d.F, d.D], F32, isOutput=False)
    out_d = nc.declare_dram_parameter("out_p", [d.T, d.D], F32, isOutput=True)
    dump_d = None
    if dbg is not None:
        dump_d = nc.declare_dram_parameter("dump", [4096, 4096], F32,
                                           isOutput=True)

    with tile.TileContext(nc) as tc:
        with ExitStack() as ctx:
            _body(ctx, tc, d, xT_d, mk_d, wq_d, wk_d, wv_d, wo_d, out_d,
                  dbg=dbg, dump_d=dump_d)
    return nc


def _split_multi_waits(nc: bass.Bass, maxw: int = 1):
    """Walrus codegen rejects instructions with more than one sync-wait
    ("Too many sync wait commands", CoreV3GenImpl setupSyncWait).  The Tile
    kernel-tail drain accumulates one wait per outstanding logical proc.
    Split any such instruction: hoist the extra waits onto same-engine no-op
    instructions inserted immediately before it (waits are AND conditions,
    so waiting sequentially is equivalent)."""
    uid = 0
    for fn in nc.m.functions:
        for bb in fn.blocks:
            insts = bb.instructions
            i = 0
            while i < len(insts):
                inst = insts[i]
                si = inst.sync_info
                if si is not None and len(si.on_wait) > maxw:
                    waits = list(si.on_wait)
                    head, keep = waits[:-maxw], waits[-maxw:]
                    for j, w in enumerate(head):
                        nop = mybir.InstNoOp(
                            name=f"bass_splitw_{uid}", engine=inst.engine,
                            ins=[], outs=[], bass_nofuse=True,
                        )
                        uid += 1
                        nop.sync_info = mybir.SyncInfo(on_wait=[w],
                                                       on_update=[])
                        insts.insert(i, nop)
                        i += 1
                    inst.sync_info = mybir.SyncInfo(on_wait=keep,
                                                    on_update=list(si.on_update))
                i += 1


def _body(ctx, tc, d: Dims, xT_d, mk_d, wq_d, wk_d, wv_d, wo_d, out_d,
          dbg=None, dump_d=None):
    nc = tc.nc
    ts = bass.ts

    persist = ctx.enter_context(tc.tile_pool(name="persist", bufs=1))
    ones_1x128 = persist.tile([1, 128], F32)
    nc.vector.memset(ones_1x128, 1.0)
    ones_128x1 = persist.tile([128, 1], F32)
    nc.vector.memset(ones_128x1, 1.0)
    ones_1x64 = persist.tile([1, 64], F32)
    nc.vector.memset(ones_1x64, 1.0)

    # aug rows + xs survive phase 1 -> phase 2 (s12 closes them before ph3)
    s12 = ctx.enter_context(ExitStack())
    aug_p = s12.enter_context(tc.tile_pool(name="aug", bufs=1, side="right"))
    aug = aug_p.tile([2, d.T], F32)    # row 0: c = -mu*rstd, row 1: ones
    # engines cannot address start-partition 1; memset both rows to 1.0,
    # row 0 is overwritten with c = -mu*rstd in phase 1
    nc.vector.memset(aug, 1.0)
    xs_p = s12.enter_context(tc.tile_pool(name="xs", bufs=d.ND, side="right"))
    xs = [xs_p.tile([128, d.T], F32, tag="xs", name=f"xs{i}")
          for i in range(d.ND)]

    # =========================================================
    # Phase 1: LN stats from streamed x^T, then xs = x^T * rstd
    # =========================================================
    with ExitStack() as s1:
        rows1 = s1.enter_context(tc.tile_pool(name="rows1", bufs=1))
        xt_p = s1.enter_context(tc.tile_pool(name="xt", bufs=3))
        sq_p = s1.enter_context(tc.tile_pool(name="sq", bufs=2))
        rb1_p = s1.enter_context(tc.tile_pool(name="rb1", bufs=d.NTC))
        ps_st = s1.enter_context(tc.tile_pool(name="ps_st", bufs=2,
                                              space="PSUM"))

        mu_row = rows1.tile([1, d.T], F32)
        m2_row = rows1.tile([1, d.T], F32)
        var_row = rows1.tile([1, d.T], F32)
        rstd_row = rows1.tile([1, d.T], F32)

        sum_ps = ps_st.tile([1, d.T], F32, tag="pss", name="sum_ps")
        msq_ps = ps_st.tile([1, d.T], F32, tag="pss", name="msq_ps")
        for dt in range(d.ND):
            xt = xt_p.tile([128, d.T], F32, tag="xt", name=f"xt{dt}")
            nc.sync.dma_start(out=xt, in_=xT_d[ts(dt, 128), :])
            sq = sq_p.tile([128, d.T], F32, tag="sq", name=f"sq{dt}")
            nc.scalar.activation(sq, xt, AF.Square)
            for tci in range(d.NTC):
                nc.tensor.matmul(
                    sum_ps[:, ts(tci, 512)], ones_128x1, xt[:, ts(tci, 512)],
                    start=(dt == 0), stop=(dt == d.ND - 1),
                )
                nc.tensor.matmul(
                    msq_ps[:, ts(tci, 512)], ones_128x1, sq[:, ts(tci, 512)],
                    start=(dt == 0), stop=(dt == d.ND - 1),
                )
        nc.scalar.activation(mu_row, sum_ps, AF.Copy, scale=1.0 / d.D)
        nc.scalar.activation(m2_row, msq_ps, AF.Copy, scale=1.0 / d.D)

        # var = (E[x^2] + eps) - mu^2 ; rstd = exp(-0.5*ln(var)) ; c = -mu*rstd
        mu2 = rows1.tile([1, d.T], F32)
        nc.vector.tensor_mul(mu2, mu_row, mu_row)
        nc.vector.scalar_tensor_tensor(
            var_row, m2_row, d.LN_EPS, mu2, ALU.add, ALU.subtract
        )
        lnv = rows1.tile([1, d.T], F32)
        nc.scalar.activation(lnv, var_row, AF.Ln)
        nc.scalar.activation(rstd_row, lnv, AF.Exp, scale=-0.5)
        nc.vector.scalar_tensor_tensor(
            aug[0:1, :], mu_row, -1.0, rstd_row, ALU.mult, ALU.mult
        )

        # broadcast rstd over 128 partitions, then xs = x^T * rstd (reload x^T)
        rb1 = []
        for tci in range(d.NTC):
            rb_ps = ps_st.tile([128, 512], F32, tag="pss", name=f"rbp{tci}")
            nc.tensor.matmul(rb_ps, ones_1x128, rstd_row[:, ts(tci, 512)],
                             start=True, stop=True)
            rb_sb = rb1_p.tile([128, 512], F32, tag="rb1", name=f"rb{tci}")
            nc.scalar.copy(rb_sb, rb_ps)
            rb1.append(rb_sb)
        for dt in range(d.ND):
            xt = xt_p.tile([128, d.T], F32, tag="xt", name=f"xr{dt}")
            nc.sync.dma_start(out=xt, in_=xT_d[ts(dt, 128), :])
            for tci in range(d.NTC):
                nc.vector.tensor_mul(
                    xs[dt][:, ts(tci, 512)], xt[:, ts(tci, 512)], rb1[tci]
                )

    if dbg == "xs":
        for dt in range(d.ND):
            nc.sync.dma_start(out=dump_d[ts(dt, 128), 0:d.T], in_=xs[dt])
        nc.sync.dma_start(out=dump_d[d.D + 1:d.D + 3, 0:d.T], in_=aug)
        return

    # =========================================================
    # Phase 2: QKV projections (order: V, Q, K to bound SBUF)
    # =========================================================
    qt_p = ctx.enter_context(tc.tile_pool(name="qt", bufs=d.NF))
    ktl_p = ctx.enter_context(tc.tile_pool(name="ktl", bufs=d.NF))
    vx_p = ctx.enter_context(tc.tile_pool(name="vx", bufs=d.NKT))
    qt = [qt_p.tile([128, d.T], F32, tag="qt", name=f"qt{i}")
          for i in range(d.NF)]
    kt = [ktl_p.tile([128, d.T], F32, tag="ktl", name=f"ktt{i}")
          for i in range(d.NF)]
    vx = [vx_p.tile([128, d.HL * 65], F32, tag="vx", name=f"vx{i}")
          for i in range(d.NKT)]

    with ExitStack() as s2:
        w_p = s2.enter_context(tc.tile_pool(name="w", bufs=d.ND + 2))
        wt_p = s2.enter_context(tc.tile_pool(name="wt", bufs=2))
        ps_qk = s2.enter_context(tc.tile_pool(name="ps_qk", bufs=3,
                                              space="PSUM"))

        def load_w(wd, pfx):
            wmain = []
            for dc in range(d.ND):
                t = w_p.tile([128, d.F], F32, tag="w", name=f"{pfx}{dc}")
                nc.sync.dma_start(out=t, in_=wd[ts(dc, 128), :])
                wmain.append(t)
            wtail = wt_p.tile([2, d.F], F32, tag="wt", name=f"{pfx}t")
            nc.sync.dma_start(out=wtail, in_=wd[d.D:d.D + 2, :])
            return wmain, wtail

        # ---- V (natural layout, interleaved 65-wide per head with ones col)
        vmain, vtail = load_w(wv_d, "wv")
        for tt in range(d.NKT):
            v_ps = ps_qk.tile([128, d.F], F32, tag="psq", name=f"vps{tt}")
            for dc in range(d.ND):
                nc.tensor.matmul(
                    v_ps, xs[dc][:, ts(tt, 128)], vmain[dc],
                    start=(dc == 0), stop=False,
                )
            nc.tensor.matmul(v_ps, aug[:, ts(tt, 128)], vtail,
                             start=False, stop=True)
            dst = vx[tt].rearrange("p (h c) -> p h c", c=65)
            src = v_ps.rearrange("p (h c) -> p h c", c=64)
            nc.vector.tensor_copy(dst[:, :, 0:64], src)
            nc.gpsimd.memset(dst[:, :, 64:65], 1.0)

        # ---- Q then K (transposed layout [feat, t])
        for wd, dest, pfx in ((wq_d, qt, "wq"), (wk_d, kt, "wk")):
            wmain, wtail = load_w(wd, pfx)
            for ft in range(d.NF):
                for tci in range(d.NTC):
                    ps = ps_qk.tile([128, 512], F32, tag="psq",
                                    name=f"{pfx}ps{ft}_{tci}")
                    for dc in range(d.ND):
                        nc.tensor.matmul(
                            ps, wmain[dc][:, ts(ft, 128)],
                            xs[dc][:, ts(tci, 512)],
                            start=(dc == 0), stop=False,
                        )
                    nc.tensor.matmul(
                        ps, wtail[:, ts(ft, 128)], aug[:, ts(tci, 512)],
                        start=False, stop=True,
                    )
                    nc.scalar.copy(dest[ft][:, ts(tci, 512)], ps)

    if dbg == "qkv":
        for ft in range(d.NF):
            nc.sync.dma_start(out=dump_d[ts(ft, 128), 0:d.T], in_=qt[ft])
            nc.sync.dma_start(out=dump_d[d.F + ft * 128:d.F + ft * 128 + 128,
                                         0:d.T], in_=kt[ft])
        for tt in range(d.NKT):
            nc.sync.dma_start(
                out=dump_d[2 * d.F + tt * 128:2 * d.F + (tt + 1) * 128,
                           0:d.HL * 65], in_=vx[tt])
        return

    # =========================================================
    # Phase 3: attention + output projection, per q-chunk
    # =========================================================
    s12.close()   # release aug + xs regions for the attention pools
    wo_p = ctx.enter_context(tc.tile_pool(name="wo", bufs=d.NF))
    mk_p = ctx.enter_context(tc.tile_pool(name="mk", bufs=d.NKT))
    ms_p = ctx.enter_context(tc.tile_pool(name="ms", bufs=3))
    avt_p = ctx.enter_context(tc.tile_pool(name="avt", bufs=d.NF + 1))
    ob_p = ctx.enter_context(tc.tile_pool(name="ob", bufs=2))
    rows3 = ctx.enter_context(tc.tile_pool(name="rows3", bufs=2))
    # PSUM: ps_a holds [128, QC] score tiles + [64, QC] norm-broadcasts,
    # ps_b holds PV accumulators [65, QC] / proj [128, 512].
    ps_a = ctx.enter_context(tc.tile_pool(name="ps_a", bufs=4, space="PSUM"))
    ps_b = ctx.enter_context(tc.tile_pool(name="ps_b", bufs=3, space="PSUM"))

    wo = []
    for ghc in range(d.NF):
        t = wo_p.tile([128, d.D], F32, tag="wo", name=f"wo{ghc}")
        nc.sync.dma_start(out=t, in_=wo_d[ts(ghc, 128), :])
        wo.append(t)

    n_qh = d.QC // 512
    for qc in range(d.NQC):
        mk = []
        for kti in range(d.NKT):
            t = mk_p.tile([128, d.QC], F32, tag="mk", name=f"mk{qc}_{kti}")
            nc.sync.dma_start(out=t, in_=mk_d[ts(kti, 128), ts(qc, d.QC)])
            mk.append(t)

        avt = [avt_p.tile([128, d.QC], F32, tag="avt", name=f"avt{qc}_{i}")
               for i in range(d.NF)]

        for h in range(d.HL):
            fti = h // 2
            po = (h % 2) * 64
            q_rhs = qt[fti][po:po + 64, ts(qc, d.QC)]
            pv_ps = ps_b.tile([65, d.QC], F32, tag="psb", name=f"pv{qc}_{h}")
            for kti in range(d.NKT):
                st = ps_a.tile([128, d.QC], F32, tag="psa",
                               name=f"st{qc}_{h}_{kti}")
                k_lhs = kt[fti][po:po + 64, ts(kti, 128)]
                for qh in range(n_qh):
                    nc.tensor.matmul(
                        st[:, ts(qh, 512)], k_lhs, q_rhs[:, ts(qh, 512)],
                        start=True, stop=True,
                    )
                ms = ms_p.tile([128, d.QC], F32, tag="ms",
                               name=f"ms{qc}_{h}_{kti}")
                nc.vector.tensor_mul(st, st, mk[kti])
                nc.scalar.activation(ms, st, AF.Exp, scale=d.SCALE)
                v_lhs = vx[kti][:, h * 65:(h + 1) * 65]
                for qh in range(n_qh):
                    nc.tensor.matmul(
                        pv_ps[:, ts(qh, 512)], v_lhs, ms[:, ts(qh, 512)],
                        start=(kti == 0), stop=(kti == d.NKT - 1),
                    )
            if dbg == "head0" and qc == 0 and h == 0:
                pv_dbg = ob_p.tile([65, d.QC], F32, tag="dbg", name="pv_dbg")
                nc.scalar.copy(pv_dbg, pv_ps)
                nc.sync.dma_start(out=dump_d[0:65, 0:d.QC], in_=pv_dbg)
                nc.sync.dma_start(out=dump_d[128:192, 0:d.QC],
                                  in_=ms[0:64, :])
            # normalize: avt_h = OT_u * exp(-ln(den)) broadcast over 64 rows
            lnd = rows3.tile([1, d.QC], F32, tag="rows3", name=f"ln{qc}_{h}")
            nc.scalar.activation(lnd, pv_ps[64:65, :], AF.Ln)
            rb = ps_a.tile([64, d.QC], F32, tag="psa", name=f"rb{qc}_{h}")
            for qh in range(n_qh):
                nc.tensor.matmul(rb[:, ts(qh, 512)], ones_1x64,
                                 lnd[:, ts(qh, 512)], start=True, stop=True)
            rbe = ms_p.tile([64, d.QC], F32, tag="ms", name=f"rbe{qc}_{h}")
            nc.scalar.activation(rbe, rb, AF.Exp, scale=-1.0)
            nc.vector.tensor_mul(avt[fti][po:po + 64, :], pv_ps[0:64, :], rbe)
            if dbg == "head0" and qc == 0 and h == 0:
                nc.sync.dma_start(out=dump_d[192:193, 0:d.QC], in_=lnd)
                nc.sync.dma_start(out=dump_d[256:320, 0:d.QC], in_=rbe)
                nc.sync.dma_start(out=dump_d[320:384, 0:d.QC],
                                  in_=avt[fti][po:po + 64, :])

        if dbg == "avt" and qc == 0:
            for ghc in range(d.NF):
                nc.sync.dma_start(out=dump_d[ts(ghc, 128), 0:d.QC],
                                  in_=avt[ghc])
            return

        # output projection for this q-chunk
        dcw = min(512, d.D)
        for tt in range(d.QC // 128):
            for dc2 in range(d.D // dcw):
                pp = ps_b.tile([128, dcw], F32, tag="psb",
                               name=f"pp{qc}_{tt}_{dc2}")
                for ghc in range(d.NF):
                    nc.tensor.matmul(
                        pp, avt[ghc][:, ts(tt, 128)], wo[ghc][:, ts(dc2, dcw)],
                        start=(ghc == 0), stop=(ghc == d.NF - 1),
                    )
                ob = ob_p.tile([128, dcw], F32, tag="ob",
                               name=f"ob{qc}_{tt}_{dc2}")
                nc.scalar.copy(ob, pp)
                nc.sync.dma_start(
                    out=out_d[qc * d.QC + tt * 128: qc * d.QC + (tt + 1) * 128,
                              ts(dc2, dcw)],
                    in_=ob,
                )


# =========================================================
# Host-side wrapper
# =========================================================
_B, _T, _D, _H, _DH = 4, 2048, 1024, 16, 64
_NCORES = 8
_CACHE = {}


def _built():
    if "nc" not in _CACHE:
        nc = build_bass(Dims())
        _split_multi_waits(nc)   # HW-compile path only; CoreSim rejects it
        _CACHE["nc"] = nc
    return _CACHE["nc"]


def _aug_w(W, gamma, beta):
    Wg = gamma[:, None] * W
    a = (gamma @ W)[None, :]
    b = (beta @ W)[None, :]
    return np.ascontiguousarray(
        np.concatenate([Wg, a, b], axis=0), dtype=np.float32
    )


def kernel(x, attn_mask, gamma, beta, Wq, Wk, Wv, Wo, bo):
    x = np.asarray(x, np.float32)
    attn_mask = np.asarray(attn_mask, np.float32)
    gamma = np.asarray(gamma, np.float32)
    beta = np.asarray(beta, np.float32)
    Wq = np.asarray(Wq, np.float32)
    Wk = np.asarray(Wk, np.float32)
    Wv = np.asarray(Wv, np.float32)
    Wo = np.asarray(Wo, np.float32)
    bo = np.asarray(bo, np.float32)

    maskT = np.ascontiguousarray(attn_mask.T)
    xTs = [np.ascontiguousarray(x[b].T) for b in range(_B)]
    F = _D // 2  # 512 features per core

    in_maps = []
    for c in range(_NCORES):
        b, hh = divmod(c, 2)
        sl = slice(hh * F, (hh + 1) * F)
        in_maps.append({
            "xT": xTs[b],
            "maskT": maskT,
            "Wq": _aug_w(Wq[:, sl], gamma, beta),
            "Wk": _aug_w(Wk[:, sl], gamma, beta),
            "Wv": _aug_w(Wv[:, sl], gamma, beta),
            "Wo": np.ascontiguousarray(Wo[sl, :]),
        })

    from concourse.bass_utils import run_bass_kernel_spmd

    res = run_bass_kernel_spmd(_built(), in_maps, list(range(_NCORES))).results
    out = np.empty((_B, _T, _D), np.float32)
    for b in range(_B):
        out[b] = res[2 * b]["out_p"] + res[2 * b + 1]["out_p"] + bo
    return out


# revision 22
# speedup vs baseline: 11.6317x; 11.6317x over previous
"""Trainium2 Bass kernel for nn_MultiHeadAttention_59992103190912.

Strategy (8 cores): data-parallel over batch (4) x tensor-parallel over heads
(2-way, 8 heads/core).  Each core computes, for its (batch b, head-half hh):

    xs   = x^T * rstd(t)                      (LayerNorm folded into weights)
    Q^T  = Wq_aug^T @ [xs; c; 1]              [512 feat, T]   (c = -mu*rstd)
    K^T  = Wk_aug^T @ [xs; c; 1]              [512 feat, T]
    V    = [xs; c; 1]^T @ Wv_aug              [T, 512 feat]   (+ ones column/head)
    S^T  = K^T_h K-tiles  x  Q^T_h            [k, q] layout per head
    Pu   = exp(SCALE * (S^T .* mask^T))       multiplicative mask, DVE mul + ACT exp
    OT_u = V_ext^T-chunks @ Pu                [65, q]; row 64 = softmax denom
    avT  = OT_u[0:64] * exp(-ln(den))         division-free normalize
    out_p= avT^T @ Wo_half                    [T, D] partial (host sums pair + bo)

All activation functions used (exp, ln, square) live in one ACT table set
(natural_log_exp_and_others) so there are no table reloads in the hot loop.
"""

import sys

sys.path.insert(0, "/opt/trn_rl_repo")

from contextlib import ExitStack
from dataclasses import dataclass

import numpy as np

import concourse.bass as bass
import concourse.tile as tile
from concourse import mybir

F32 = mybir.dt.float32
F32R = mybir.dt.float32r   # fp32 "rounded" matmul format: 1 PE cycle/row
AF = mybir.ActivationFunctionType
ALU = mybir.AluOpType


@dataclass(frozen=True)
class Dims:
    T: int = 2048      # sequence length
    D: int = 1024      # model dim
    HL: int = 8        # heads per core
    DH: int = 64       # head dim
    QC: int = 512      # q-chunk width for attention phase
    LN_EPS: float = 1e-5

    @property
    def F(self):       # features per core (= HL * DH)
        return self.HL * self.DH

    @property
    def ND(self):      # d-chunks of 128
        return self.D // 128

    @property
    def NF(self):      # feature tiles of 128
        return self.F // 128

    @property
    def NKT(self):     # k tiles of 128
        return self.T // 128

    @property
    def NQC(self):     # q chunks
        return self.T // self.QC

    @property
    def NTC(self):     # 512-wide t-chunks (QKV phase)
        return self.T // 512

    @property
    def SCALE(self):
        return self.DH ** -0.5


def build_bass(d: Dims = Dims(), dbg: str | None = None) -> bass.Bass:
    assert d.T % 512 == 0 and d.D % 128 == 0 and d.F % 128 == 0
    assert d.QC % 512 == 0 and d.T % d.QC == 0
    nc = bass.Bass()

    xT_d = nc.declare_dram_parameter("xT", [d.D, d.T], F32R, isOutput=False)
    mk_d = nc.declare_dram_parameter("maskT", [d.T, d.T], F32, isOutput=False)
    wq_d = nc.declare_dram_parameter("Wq", [d.D + 2, d.F], F32R, isOutput=False)
    wk_d = nc.declare_dram_parameter("Wk", [d.D + 2, d.F], F32R, isOutput=False)
    wv_d = nc.declare_dram_parameter("Wv", [d.D + 2, d.F], F32R, isOutput=False)
    wo_d = nc.declare_dram_parameter("Wo", [d.F, d.D], F32R, isOutput=False)
    out_d = nc.declare_dram_parameter("out_p", [d.T, d.D], F32, isOutput=True)
    dump_d = None
    if dbg is not None:
        dump_d = nc.declare_dram_parameter("dump", [4096, 4096], F32,
                                           isOutput=True)

    with tile.TileContext(nc) as tc:
        with ExitStack() as ctx:
            _body(ctx, tc, d, xT_d, mk_d, wq_d, wk_d, wv_d, wo_d, out_d,
                  dbg=dbg, dump_d=dump_d)
    return nc


def _split_multi_waits(nc: bass.Bass, maxw: int = 1):
    """Walrus codegen rejects instructions with more than one sync-wait
    ("Too many sync wait commands", CoreV3GenImpl setupSyncWait).  The Tile
    kernel-tail drain accumulates one wait per outstanding logical proc.
    Split any such instruction: hoist the extra waits onto same-engine no-op
    instructions inserted immediately before it (waits are AND conditions,
    so waiting sequentially is equivalent)."""
    uid = 0
    for fn in nc.m.functions:
        for bb in fn.blocks:
            insts = bb.instructions
            i = 0
            while i < len(insts):
                inst = insts[i]
                si = inst.sync_info
                if si is not None and len(si.on_wait) > maxw:
                    waits = list(si.on_wait)
                    head, keep = waits[:-maxw], waits[-maxw:]
                    for j, w in enumerate(head):
                        nop = mybir.InstNoOp(
                            name=f"bass_splitw_{uid}", engine=inst.engine,
                            ins=[], outs=[], bass_nofuse=True,
                        )
                        uid += 1
                        nop.sync_info = mybir.SyncInfo(on_wait=[w],
                                                       on_update=[])
                        insts.insert(i, nop)
                        i += 1
                    inst.sync_info = mybir.SyncInfo(on_wait=keep,
                                                    on_update=list(si.on_update))
                i += 1


def _body(ctx, tc, d: Dims, xT_d, mk_d, wq_d, wk_d, wv_d, wo_d, out_d,
          dbg=None, dump_d=None):
    nc = tc.nc
    ts = bass.ts

    persist = ctx.enter_context(tc.tile_pool(name="persist", bufs=1))
    # memset cannot produce float32r; DVE copy-converts from f32 scratch
    ones_f = persist.tile([128, 128], F32)
    nc.vector.memset(ones_f, 1.0)
    ones_1x128 = persist.tile([1, 128], F32R)
    nc.vector.tensor_copy(ones_1x128, ones_f[0:1, 0:128])
    ones_128x1 = persist.tile([128, 1], F32R)
    nc.vector.tensor_copy(ones_128x1, ones_f[:, 0:1])
    ones_1x64 = persist.tile([1, 64], F32R)
    nc.vector.tensor_copy(ones_1x64, ones_f[0:1, 0:64])

    # aug rows + xs survive phase 1 -> phase 2 (s12 closes them before ph3)
    s12 = ctx.enter_context(ExitStack())
    aug_p = s12.enter_context(tc.tile_pool(name="aug", bufs=1, side="right"))
    aug_f = aug_p.tile([2, d.T], F32)  # row 0: c = -mu*rstd, row 1: ones
    # engines cannot address start-partition 1; memset both rows to 1.0,
    # row 0 is overwritten with c = -mu*rstd in phase 1
    nc.vector.memset(aug_f, 1.0)
    aug = aug_p.tile([2, d.T], F32R)   # rounded copy, made after phase 1
    xs_p = s12.enter_context(tc.tile_pool(name="xs", bufs=d.ND, side="right"))
    xs = [xs_p.tile([128, d.T], F32R, tag="xs", name=f"xs{i}")
          for i in range(d.ND)]

    # =========================================================
    # Phase 1: LN stats from streamed x^T, then xs = x^T * rstd
    # =========================================================
    with ExitStack() as s1:
        rows1 = s1.enter_context(tc.tile_pool(name="rows1", bufs=1))
        xt_p = s1.enter_context(tc.tile_pool(name="xt", bufs=d.ND))
        sq_p = s1.enter_context(tc.tile_pool(name="sq", bufs=2))
        rb1_p = s1.enter_context(tc.tile_pool(name="rb1", bufs=d.NTC))
        ps_st = s1.enter_context(tc.tile_pool(name="ps_st", bufs=2,
                                              space="PSUM"))

        mu_row = rows1.tile([1, d.T], F32)
        m2_row = rows1.tile([1, d.T], F32, tag="m2lnv", name="m2_row")
        var_row = rows1.tile([1, d.T], F32)
        rstd_row = rows1.tile([1, d.T], F32R, tag="mu2rstd", name="rstd_row")

        sum_ps = ps_st.tile([1, d.T], F32, tag="pss", name="sum_ps")
        msq_ps = ps_st.tile([1, d.T], F32, tag="pss", name="msq_ps")
        xts = []
        for dt in range(d.ND):
            xt = xt_p.tile([128, d.T], F32R, tag="xt", name=f"xt{dt}")
            nc.sync.dma_start(out=xt, in_=xT_d[ts(dt, 128), :])
            xts.append(xt)
            sq = sq_p.tile([128, d.T], F32R, tag="sq", name=f"sq{dt}")
            nc.scalar.activation(sq, xt.bitcast(F32), AF.Square)
            for tci in range(d.NTC):
                nc.tensor.matmul(
                    sum_ps[:, ts(tci, 512)], ones_128x1, xt[:, ts(tci, 512)],
                    start=(dt == 0), stop=(dt == d.ND - 1),
                )
                nc.tensor.matmul(
                    msq_ps[:, ts(tci, 512)], ones_128x1, sq[:, ts(tci, 512)],
                    start=(dt == 0), stop=(dt == d.ND - 1),
                )
        nc.scalar.activation(mu_row, sum_ps, AF.Copy, scale=1.0 / d.D)
        nc.scalar.activation(m2_row, msq_ps, AF.Copy, scale=1.0 / d.D)

        # var = (E[x^2] + eps) - mu^2 ; rstd = exp(-0.5*ln(var)) ; c = -mu*rstd
        mu2 = rows1.tile([1, d.T], F32, tag="mu2rstd", name="mu2")
        nc.vector.tensor_mul(mu2, mu_row, mu_row)
        nc.vector.scalar_tensor_tensor(
            var_row, m2_row, d.LN_EPS, mu2, ALU.add, ALU.subtract
        )
        lnv = rows1.tile([1, d.T], F32, tag="m2lnv", name="lnv")
        nc.scalar.activation(lnv, var_row, AF.Ln)
        nc.scalar.activation(rstd_row, lnv, AF.Exp, scale=-0.5)
        nc.vector.scalar_tensor_tensor(
            aug_f[0:1, :], mu_row, -1.0, rstd_row.bitcast(F32),
            ALU.mult, ALU.mult
        )
        nc.vector.tensor_copy(aug, aug_f)

        # broadcast rstd over 128 partitions, then xs = x^T * rstd (reload x^T)
        rb1 = []
        for tci in range(d.NTC):
            rb_ps = ps_st.tile([128, 512], F32, tag="pss", name=f"rbp{tci}")
            nc.tensor.matmul(rb_ps, ones_1x128, rstd_row[:, ts(tci, 512)],
                             start=True, stop=True)
            rb_sb = rb1_p.tile([128, 512], F32, tag="rb1", name=f"rb{tci}")
            nc.scalar.copy(rb_sb, rb_ps)
            rb1.append(rb_sb)
        for dt in range(d.ND):
            for tci in range(d.NTC):
                nc.vector.tensor_mul(
                    xs[dt][:, ts(tci, 512)],
                    xts[dt].bitcast(F32)[:, ts(tci, 512)], rb1[tci]
                )

    if dbg == "xs":
        for dt in range(d.ND):
            nc.sync.dma_start(out=dump_d[ts(dt, 128), 0:d.T], in_=xs[dt].bitcast(F32))
        nc.sync.dma_start(out=dump_d[d.D + 1:d.D + 3, 0:d.T], in_=aug.bitcast(F32))
        return

    # =========================================================
    # Phase 2: QKV projections (order: V, Q, K to bound SBUF)
    # =========================================================
    qt_p = ctx.enter_context(tc.tile_pool(name="qt", bufs=d.NF))
    ktl_p = ctx.enter_context(tc.tile_pool(name="ktl", bufs=d.NF))
    vx_p = ctx.enter_context(tc.tile_pool(name="vx", bufs=d.NKT))
    qt = [qt_p.tile([128, d.T], F32R, tag="qt", name=f"qt{i}")
          for i in range(d.NF)]
    kt = [ktl_p.tile([128, d.T], F32R, tag="ktl", name=f"ktt{i}")
          for i in range(d.NF)]
    vx = [vx_p.tile([128, d.HL * 65], F32R, tag="vx", name=f"vx{i}")
          for i in range(d.NKT)]

    with ExitStack() as s2:
        w_p = s2.enter_context(tc.tile_pool(name="w", bufs=d.ND + 2))
        wt_p = s2.enter_context(tc.tile_pool(name="wt", bufs=2))
        ps_qk = s2.enter_context(tc.tile_pool(name="ps_qk", bufs=3,
                                              space="PSUM"))

        def load_w(wd, pfx):
            wmain = []
            for dc in range(d.ND):
                t = w_p.tile([128, d.F], F32R, tag="w", name=f"{pfx}{dc}")
                nc.sync.dma_start(out=t, in_=wd[ts(dc, 128), :])
                wmain.append(t)
            wtail = wt_p.tile([2, d.F], F32R, tag="wt", name=f"{pfx}t")
            nc.sync.dma_start(out=wtail, in_=wd[d.D:d.D + 2, :])
            return wmain, wtail

        # ---- V (natural layout, interleaved 65-wide per head with ones col)
        vmain, vtail = load_w(wv_d, "wv")
        for tt in range(d.NKT):
            v_ps = ps_qk.tile([128, d.F], F32, tag="psq", name=f"vps{tt}")
            for dc in range(d.ND):
                nc.tensor.matmul(
                    v_ps, xs[dc][:, ts(tt, 128)], vmain[dc],
                    start=(dc == 0), stop=False,
                )
            nc.tensor.matmul(v_ps, aug[:, ts(tt, 128)], vtail,
                             start=False, stop=True)
            dst = vx[tt].rearrange("p (h c) -> p h c", c=65)
            src = v_ps.rearrange("p (h c) -> p h c", c=64)
            nc.vector.tensor_copy(dst[:, :, 0:64], src)
            nc.vector.tensor_copy(dst[:, :, 64], ones_f[:, 0:d.HL])

        # ---- Q then K (transposed layout [feat, t])
        for wd, dest, pfx in ((wq_d, qt, "wq"), (wk_d, kt, "wk")):
            wmain, wtail = load_w(wd, pfx)
            for ft in range(d.NF):
                for tci in range(d.NTC):
                    ps = ps_qk.tile([128, 512], F32, tag="psq",
                                    name=f"{pfx}ps{ft}_{tci}")
                    for dc in range(d.ND):
                        nc.tensor.matmul(
                            ps, wmain[dc][:, ts(ft, 128)],
                            xs[dc][:, ts(tci, 512)],
                            start=(dc == 0), stop=False,
                        )
                    nc.tensor.matmul(
                        ps, wtail[:, ts(ft, 128)], aug[:, ts(tci, 512)],
                        start=False, stop=True,
                    )
                    nc.scalar.copy(dest[ft][:, ts(tci, 512)], ps)

    if dbg == "qkv":
        for ft in range(d.NF):
            nc.sync.dma_start(out=dump_d[ts(ft, 128), 0:d.T], in_=qt[ft].bitcast(F32))
            nc.sync.dma_start(out=dump_d[d.F + ft * 128:d.F + ft * 128 + 128,
                                         0:d.T], in_=kt[ft].bitcast(F32))
        for tt in range(d.NKT):
            nc.sync.dma_start(
                out=dump_d[2 * d.F + tt * 128:2 * d.F + (tt + 1) * 128,
                           0:d.HL * 65], in_=vx[tt].bitcast(F32))
        return

    # =========================================================
    # Phase 3: attention + output projection, per q-chunk
    # =========================================================
    s12.close()   # release aug + xs regions for the attention pools
    wo_p = ctx.enter_context(tc.tile_pool(name="wo", bufs=d.NF))
    mk_p = ctx.enter_context(tc.tile_pool(name="mk", bufs=d.NKT // 2))
    ms_p = ctx.enter_context(tc.tile_pool(name="ms", bufs=3))
    msm_p = ctx.enter_context(tc.tile_pool(name="msm", bufs=3))
    avt_p = ctx.enter_context(tc.tile_pool(name="avt", bufs=d.NF + 1))
    ob_p = ctx.enter_context(tc.tile_pool(name="ob", bufs=2))
    rows3 = ctx.enter_context(tc.tile_pool(name="rows3", bufs=2))
    # PSUM: ps_a holds [128, QC] score tiles + [64, QC] norm-broadcasts,
    # ps_b holds PV accumulators [65, QC] / proj [128, 512].
    ps_a = ctx.enter_context(tc.tile_pool(name="ps_a", bufs=2, space="PSUM"))
    ps_b = ctx.enter_context(tc.tile_pool(name="ps_b", bufs=4, space="PSUM"))

    wo = []
    for ghc in range(d.NF):
        t = wo_p.tile([128, d.D], F32R, tag="wo", name=f"wo{ghc}")
        nc.sync.dma_start(out=t, in_=wo_d[ts(ghc, 128), :])
        wo.append(t)

    n_qh = d.QC // 512
    for qc in range(d.NQC):
        mk = []
        for j in range(d.NKT // 2):
            t = mk_p.tile([128, 2 * d.QC], F32, tag="mk", name=f"mk{qc}_{j}")
            nc.sync.dma_start(out=t[:, 0:d.QC],
                              in_=mk_d[ts(2 * j, 128), ts(qc, d.QC)])
            nc.sync.dma_start(out=t[:, d.QC:2 * d.QC],
                              in_=mk_d[ts(2 * j + 1, 128), ts(qc, d.QC)])
            mk.append(t)

        avt = [avt_p.tile([128, d.QC], F32R, tag="avt", name=f"avt{qc}_{i}")
               for i in range(d.NF)]

        for h in range(d.HL):
            fti = h // 2
            po = (h % 2) * 64
            q_rhs = qt[fti][po:po + 64, ts(qc, d.QC)]
            pv_ps = ps_b.tile([65, d.QC], F32, tag="psb", name=f"pv{qc}_{h}")
            for j in range(d.NKT // 2):
                # two k-tiles share one [128, 2*QC] PSUM tile: same q rhs,
                # different K weights; halves the DVE/ACT per-op overhead
                st = ps_a.tile([128, 2 * d.QC], F32, tag="psa",
                               name=f"st{qc}_{h}_{j}")
                for half in range(2):
                    kti = 2 * j + half
                    k_lhs = kt[fti][po:po + 64, ts(kti, 128)]
                    for qh in range(n_qh):
                        nc.tensor.matmul(
                            st[:, half * d.QC + qh * 512:
                               half * d.QC + (qh + 1) * 512],
                            k_lhs, q_rhs[:, ts(qh, 512)],
                            start=True, stop=True,
                        )
                msm = msm_p.tile([128, 2 * d.QC], F32, tag="msm",
                                 name=f"msm{qc}_{h}_{j}")
                nc.vector.tensor_mul(msm, st, mk[j])
                ms = ms_p.tile([128, 2 * d.QC], F32R, tag="ms",
                               name=f"ms{qc}_{h}_{j}")
                nc.scalar.activation(ms, msm, AF.Exp, scale=d.SCALE)
                for half in range(2):
                    kti = 2 * j + half
                    v_lhs = vx[kti][:, h * 65:(h + 1) * 65]
                    for qh in range(n_qh):
                        nc.tensor.matmul(
                            pv_ps[:, ts(qh, 512)], v_lhs,
                            ms[:, half * d.QC + qh * 512:
                               half * d.QC + (qh + 1) * 512],
                            start=(kti == 0), stop=(kti == d.NKT - 1),
                        )
            if dbg == "head0" and qc == 0 and h == 0:
                pv_dbg = ob_p.tile([65, d.QC], F32, tag="dbg", name="pv_dbg")
                nc.scalar.copy(pv_dbg, pv_ps)
                nc.sync.dma_start(out=dump_d[0:65, 0:d.QC], in_=pv_dbg)
                nc.sync.dma_start(out=dump_d[128:192, 0:d.QC],
                                  in_=ms.bitcast(F32)[0:64, :])
            # normalize: avt_h = OT_u * exp(-ln(den)) broadcast over 64 rows
            lnd = rows3.tile([1, d.QC], F32R, tag="rows3", name=f"ln{qc}_{h}")
            nc.scalar.activation(lnd, pv_ps[64:65, :], AF.Ln)
            rb = ps_b.tile([64, d.QC], F32, tag="psb", name=f"rb{qc}_{h}")
            for qh in range(n_qh):
                nc.tensor.matmul(rb[:, ts(qh, 512)], ones_1x64,
                                 lnd[:, ts(qh, 512)], start=True, stop=True)
            rbe = ms_p.tile([64, d.QC], F32, tag="ms", name=f"rbe{qc}_{h}")
            nc.scalar.activation(rbe, rb, AF.Exp, scale=-1.0)
            nc.vector.tensor_mul(avt[fti][po:po + 64, :], pv_ps[0:64, :], rbe)
            if dbg == "head0" and qc == 0 and h == 0:
                nc.sync.dma_start(out=dump_d[192:193, 0:d.QC], in_=lnd.bitcast(F32))
                nc.sync.dma_start(out=dump_d[256:320, 0:d.QC], in_=rbe)
                nc.sync.dma_start(out=dump_d[320:384, 0:d.QC],
                                  in_=avt[fti].bitcast(F32)[po:po + 64, :])

        if dbg == "avt" and qc == 0:
            for ghc in range(d.NF):
                nc.sync.dma_start(out=dump_d[ts(ghc, 128), 0:d.QC],
                                  in_=avt[ghc].bitcast(F32))
            return

        # output projection for this q-chunk
        dcw = min(512, d.D)
        for tt in range(d.QC // 128):
            for dc2 in range(d.D // dcw):
                pp = ps_b.tile([128, dcw], F32, tag="psb",
                               name=f"pp{qc}_{tt}_{dc2}")
                for ghc in range(d.NF):
                    nc.tensor.matmul(
                        pp, avt[ghc][:, ts(tt, 128)], wo[ghc][:, ts(dc2, dcw)],
                        start=(ghc == 0), stop=(ghc == d.NF - 1),
                    )
                ob = ob_p.tile([128, dcw], F32, tag="ob",
                               name=f"ob{qc}_{tt}_{dc2}")
                nc.scalar.copy(ob, pp)
                nc.sync.dma_start(
                    out=out_d[qc * d.QC + tt * 128: qc * d.QC + (tt + 1) * 128,
                              ts(dc2, dcw)],
                    in_=ob,
                )


# =========================================================
# Host-side wrapper
# =========================================================
_B, _T, _D, _H, _DH = 4, 2048, 1024, 16, 64
_NCORES = 8
_CACHE = {}


def _built():
    if "nc" not in _CACHE:
        nc = build_bass(Dims())
        _split_multi_waits(nc)   # HW-compile path only; CoreSim rejects it
        _CACHE["nc"] = nc
    return _CACHE["nc"]


def _aug_w(W, gamma, beta):
    Wg = gamma[:, None] * W
    a = (gamma @ W)[None, :]
    b = (beta @ W)[None, :]
    return np.ascontiguousarray(
        np.concatenate([Wg, a, b], axis=0), dtype=np.float32
    )


def kernel(x, attn_mask, gamma, beta, Wq, Wk, Wv, Wo, bo):
    x = np.asarray(x, np.float32)
    attn_mask = np.asarray(attn_mask, np.float32)
    gamma = np.asarray(gamma, np.float32)
    beta = np.asarray(beta, np.float32)
    Wq = np.asarray(Wq, np.float32)
    Wk = np.asarray(Wk, np.float32)
    Wv = np.asarray(Wv, np.float32)
    Wo = np.asarray(Wo, np.float32)
    bo = np.asarray(bo, np.float32)

    maskT = np.ascontiguousarray(attn_mask.T)
    xTs = [np.ascontiguousarray(x[b].T) for b in range(_B)]
    F = _D // 2  # 512 features per core

    in_maps = []
    for c in range(_NCORES):
        b, hh = divmod(c, 2)
        sl = slice(hh * F, (hh + 1) * F)
        in_maps.append({
            "xT": xTs[b],
            "maskT": maskT,
            "Wq": _aug_w(Wq[:, sl], gamma, beta),
            "Wk": _aug_w(Wk[:, sl], gamma, beta),
            "Wv": _aug_w(Wv[:, sl], gamma, beta),
            "Wo": np.ascontiguousarray(Wo[sl, :]),
        })

    from concourse.bass_utils import run_bass_kernel_spmd

    res = run_bass_kernel_spmd(_built(), in_maps, list(range(_NCORES))).results
    out = np.empty((_B, _T, _D), np.float32)
    for b in range(_B):
        out[b] = res[2 * b]["out_p"] + res[2 * b + 1]["out_p"] + bo
    return out
